# revision 1
# baseline (speedup 1.0000x reference)
"""Cubic B-spline basis expansion on Trainium2, SPMD across 8 NeuronCores.

Problem: xs [131072] f32, B [4,4] f32 (ascending-power coeffs), n=2048, q=3.
Output [131072, 2048] f32: each row i is zeros except 4 contiguous values at
columns first_i..first_i+3 where first_i = floor(xs[i]) (H=1, T0=0), and
value[k] = sum_p (frac + (q-k))^p * B[k,p].

Strategy (data-parallel, no cross-core comms):
  - shard xs / output rows across 8 cores (16384 rows each)
  - per core: bulk zero-fill the 128 MiB output shard via 2 MiB HWDGE DMAs
    (single SP ring, sequential address stream) from a zeroed SBUF tile,
    while DVE computes the 4 values + a per-row element offset; then SWDGE
    indirect-DMA scatters the 16-byte value groups into the zeroed DRAM.
  - rows are laid out j-major (row = j*128 + p) so that scatter call j
    (one index per partition, the HW-native indirect granularity) covers a
    contiguous 128-row block, letting scatters chase the zero-fill with
    per-chunk semaphore ordering.
Measured ~355-400 us per core steady-state (in-NEFF iteration slope), i.e.
at the ~358 GB/s per-NeuronCore HBM write roofline for the 128 MiB shard.
"""
import sys

import numpy as np

for _p in ("/opt/trn_rl_repo",):
    if _p not in sys.path:
        sys.path.insert(0, _p)

import concourse.bass as bass
import concourse.mybir as mybir
from concourse.bass_utils import run_bass_kernel_spmd

# Problem constants (hardcoded per contract)
NS = 131072           # total samples
N = 2048              # knots (output columns)
Q = 3                 # spline order
NCORES = 8
R = NS // NCORES      # 16384 rows per core
P = 128               # SBUF partitions
J = R // P            # 128 row-blocks (scatter calls) per core
NCHUNK = 16           # ordering chunks
JC = J // NCHUNK      # 8 scatter calls per chunk
ZFREE = 4096          # zero tile free dim (f32): 2 MiB tile
ZROWS_PER_DMA = P * ZFREE // N    # 512 output rows zeroed per DMA
NZDMA = R // ZROWS_PER_DMA        # 32 zero DMAs
ZDMA_PER_CHUNK = NZDMA // NCHUNK  # 4

F32 = mybir.dt.float32
I32 = mybir.dt.int32
ALU = mybir.AluOpType


def _build(B_np: np.ndarray, iters: int = 1, zfree: int = ZFREE,
           rings: int = 1, nchunk: int = NCHUNK) -> bass.Bass:
    # rings=1 measured ~20 us/iter faster than rings=2: one sequential
    # HBM write stream beats two interleaved streams from SP+ACT rings.
    # iters > 1 repeats the zero-fill + scatter phase (idempotent) inside
    # one NEFF — used only by the timing harness to measure per-iteration
    # HW time as a slope, cancelling dispatch overhead.
    zrows_per_dma = P * zfree // N
    nzdma = R // zrows_per_dma
    zdma_per_chunk = nzdma // nchunk
    jc = J // nchunk
    assert zdma_per_chunk * nchunk == nzdma and zdma_per_chunk >= 1
    assert jc * nchunk == J
    nc = bass.Bass("TRN2")
    xs_d = nc.dram_tensor("xs", [P, J], F32, kind="ExternalInput")
    ib_d = nc.dram_tensor("ibase", [P, J], I32, kind="ExternalInput")
    out_d = nc.dram_tensor("out", [R, N], F32, kind="ExternalOutput")

    Bc = np.asarray(B_np, dtype=np.float64)  # [Q+1, Q+1], ascending powers

    from contextlib import ExitStack

    with (
        nc.sbuf_tensor("zt", [P, zfree], F32) as zt,
        nc.sbuf_tensor("xs_t", [P, J], F32) as xs_t,
        nc.sbuf_tensor("ib_t", [P, J], I32) as ib_t,
        nc.sbuf_tensor("fi_f", [P, J], F32) as fi_f,
        nc.sbuf_tensor("gt_t", [P, J], F32) as gt_t,
        nc.sbuf_tensor("frac", [P, J], F32) as frac,
        nc.sbuf_tensor("xl", [P, J], F32) as xl,
        nc.sbuf_tensor("hh", [P, J], F32) as hh,
        nc.sbuf_tensor("fi_i", [P, J], I32) as fi_i,
        nc.sbuf_tensor("idx", [P, J], I32) as idx,
        nc.sbuf_tensor("vals", [P, (Q + 1) * J], F32) as vals,
        nc.semaphore("msem") as msem,
        nc.semaphore("xsem") as xsem,
        nc.semaphore("csem") as csem,
        nc.semaphore("ssem") as ssem,
        nc.semaphore("vsem") as vsem,
        ExitStack() as es,
    ):
        zsems = [es.enter_context(nc.semaphore(f"zsem{c}")) for c in range(nchunk)]

        with nc.Block() as block:

            @block.vector
            def _(v):
                # DVE ops are chained through vsem: deep engine pipelines mean
                # same-engine RAW hazards still need semaphore sync.
                nv = 0

                def step(inst):
                    nonlocal nv
                    inst.then_inc(vsem, 1)
                    nv += 1

                def fence():
                    v.wait_ge(vsem, nv)

                v.memset(zt[:], 0.0).then_inc(msem, 1)
                v.wait_ge(xsem, 32)
                # first_i = floor(xs) for xs >= 0, robust to any f32->i32
                # rounding mode: convert, round-trip, subtract 1 where the
                # round-trip exceeded xs.
                step(v.tensor_copy(out=fi_i[:], in_=xs_t[:]))
                fence()
                step(v.tensor_copy(out=fi_f[:], in_=fi_i[:]))
                fence()
                step(v.tensor_tensor(out=gt_t[:], in0=fi_f[:], in1=xs_t[:],
                                     op=ALU.is_gt))
                fence()
                step(v.tensor_tensor(out=fi_f[:], in0=fi_f[:], in1=gt_t[:],
                                     op=ALU.subtract))
                fence()
                step(v.tensor_tensor(out=frac[:], in0=xs_t[:], in1=fi_f[:],
                                     op=ALU.subtract))
                step(v.tensor_copy(out=fi_i[:], in_=fi_f[:]))
                fence()
                # idx = p*N + first_i (< 2^24 so the f32 ALU path is exact)
                step(v.tensor_tensor(out=idx[:], in0=ib_t[:], in1=fi_i[:],
                                     op=ALU.add))
                # values[k] = Horner(B[k], frac + (Q-k)), written interleaved
                # so vals[p, 4j+k] = value_k(row j*128+p)
                vv = vals[:].rearrange("p (j k) -> p j k", k=Q + 1)
                for k in range(Q + 1):
                    b3, b2, b1, b0 = (float(Bc[k, 3]), float(Bc[k, 2]),
                                      float(Bc[k, 1]), float(Bc[k, 0]))
                    fence()
                    step(v.tensor_scalar(out=xl[:], in0=frac[:],
                                         scalar1=float(Q - k),
                                         scalar2=None, op0=ALU.add))
                    fence()
                    step(v.tensor_scalar(out=hh[:], in0=xl[:], scalar1=b3,
                                         scalar2=b2,
                                         op0=ALU.mult, op1=ALU.add))
                    fence()
                    step(v.tensor_tensor(out=hh[:], in0=hh[:], in1=xl[:],
                                         op=ALU.mult))
                    fence()
                    step(v.tensor_scalar(out=hh[:], in0=hh[:], scalar1=b1,
                                         scalar2=None, op0=ALU.add))
                    fence()
                    step(v.tensor_tensor(out=hh[:], in0=hh[:], in1=xl[:],
                                         op=ALU.mult))
                    fence()
                    step(v.tensor_scalar(out=vv[:, :, k], in0=hh[:], scalar1=b0,
                                         scalar2=None, op0=ALU.add))
                fence()
                v.sem_inc(csem, 1)

            @block.sync
            def _(s):
                s.wait_ge(msem, 1)
                for _it in range(iters):
                    for i in range(0, nzdma, rings):
                        s.dma_start(
                            out=out_d[i * zrows_per_dma:(i + 1) * zrows_per_dma, :],
                            in_=zt[:],
                        ).then_inc(zsems[i // zdma_per_chunk], 16)

            if rings == 2:
                @block.scalar
                def _(s):
                    s.wait_ge(msem, 1)
                    for _it in range(iters):
                        for i in range(1, nzdma, 2):
                            s.dma_start(
                                out=out_d[i * zrows_per_dma:(i + 1) * zrows_per_dma, :],
                                in_=zt[:],
                            ).then_inc(zsems[i // zdma_per_chunk], 16)

            @block.gpsimd
            def _(g):
                g.dma_start(out=xs_t[:], in_=xs_d[:]).then_inc(xsem, 16)
                g.dma_start(out=ib_t[:], in_=ib_d[:]).then_inc(xsem, 16)
                g.wait_ge(csem, 1)
                for it in range(iters):
                    for c in range(nchunk):
                        g.wait_ge(zsems[c], 16 * zdma_per_chunk * (it + 1))
                        for j in range(c * jc, (c + 1) * jc):
                            # one index per partition (HW-native granularity):
                            # writes vals[p, 4j:4j+4] at element
                            # j*P*N + idx[p, j] = (j*P + p)*N + first_i
                            g.indirect_dma_start(
                                out=out_d[:],
                                out_offset=bass.IndirectOffsetOnAxis(
                                    ap=idx[:, j:j + 1], axis=1),
                                in_=vals[:, (Q + 1) * j:(Q + 1) * (j + 1)],
                                in_offset=None,
                                element_offset=j * P * N,
                            ).then_inc(ssem, 16)
                g.wait_ge(ssem, 16 * J * iters)

    return nc


_CACHE: dict[bytes, bass.Bass] = {}


def _get_program(B: np.ndarray) -> bass.Bass:
    key = np.asarray(B, dtype=np.float32).tobytes()
    if key not in _CACHE:
        _CACHE[key] = _build(B)
    return _CACHE[key]


def _in_maps(xs: np.ndarray) -> list[dict[str, np.ndarray]]:
    # j-major row layout: xs2d[p, j] = xs_shard[j*P + p]; row base offset
    # within a 128-row block is p*N (< 2^24 so DVE f32-ALU int math is
    # exact); the block base j*P*N goes in via indirect-DMA element_offset.
    ibase = np.broadcast_to(
        (np.arange(P, dtype=np.int32) * N)[:, None], (P, J)).copy()
    maps = []
    for c in range(NCORES):
        shard = np.asarray(xs[c * R:(c + 1) * R], dtype=np.float32)
        xs2d = np.ascontiguousarray(shard.reshape(J, P).T)
        maps.append({"xs": xs2d, "ibase": ibase})
    return maps


def kernel(xs, B, n, q):
    xs = np.asarray(xs, dtype=np.float32)
    B = np.asarray(B, dtype=np.float32)
    n = int(np.asarray(n)) if not isinstance(n, int) else n
    q = int(np.asarray(q)) if not isinstance(q, int) else q
    assert xs.shape == (NS,), xs.shape
    assert B.shape == (Q + 1, Q + 1), B.shape
    assert n == N and q == Q, (n, q)

    nc = _get_program(B)
    try:
        res = run_bass_kernel_spmd(nc, _in_maps(xs), core_ids=list(range(NCORES)))
    except Exception:
        # one retry for transient device-state errors (e.g. a wedged core
        # left over from a previous process)
        res = run_bass_kernel_spmd(nc, _in_maps(xs), core_ids=list(range(NCORES)))
    return np.concatenate([res.results[c]["out"] for c in range(NCORES)], axis=0)



# revision 3
# speedup vs baseline: 2.3157x; 2.3157x over previous
"""Cubic B-spline basis expansion on Trainium2, SPMD across 8 NeuronCores.

Problem: xs [131072] f32, B [4,4] f32 (ascending-power coeffs), n=2048, q=3.
Output [131072, 2048] f32: each row i is zeros except 4 contiguous values at
columns first_i..first_i+3 where first_i = floor(xs[i]) (H=1, T0=0), and
value[k] = sum_p (frac + (q-k))^p * B[k,p].

Strategy (data-parallel, no cross-core comms):
  - shard xs / output rows across 8 cores (16384 rows each)
  - the bass runtime pre-zeros ExternalOutput buffers on both execution
    paths (bass_utils native run_neff and bass2jax PJRT donation — see the
    "kernels that don't write every element rely on that" contract in
    bass2jax.run_bass_via_pjrt), so the kernel never writes the zeros:
    it only scatters the 4 nonzero values per row (256 KiB/core instead
    of the 128 MiB dense shard).
  - per core: DVE computes the 4 polynomial values + a per-row element
    offset; gpsimd then issues NG grouped SWDGE indirect DMAs, each
    carrying P*GJ row offsets with 16 B of payload per row.
  - rows are laid out j-major (row = j*128 + p); each scatter group g
    covers j in [g*GJ, (g+1)*GJ) with element_offset = g*GJ*P*N so every
    on-device index stays < 2^23 (exact under the DVE f32 ALU path).
"""
import sys

import numpy as np

for _p in ("/opt/trn_rl_repo",):
    if _p not in sys.path:
        sys.path.insert(0, _p)

import concourse.bass as bass
import concourse.mybir as mybir
from concourse.bass_utils import run_bass_kernel_spmd

# Problem constants (hardcoded per contract)
NS = 131072           # total samples
N = 2048              # knots (output columns)
Q = 3                 # spline order
NCORES = 8
R = NS // NCORES      # 16384 rows per core
P = 128               # SBUF partitions
J = R // P            # 128 row-blocks per core
NG = 128              # scatter groups (indirect DMA calls) per iteration
                      # HW contract: one offset per partition per call,
                      # whole in_ row written contiguously at that offset
GJ = J // NG          # j-blocks per scatter group

F32 = mybir.dt.float32
I32 = mybir.dt.int32
ALU = mybir.AluOpType


def _build(B_np: np.ndarray, iters: int = 1, ng: int = NG) -> bass.Bass:
    # iters > 1 repeats the compute+scatter phase (idempotent) inside one
    # NEFF — used only by the timing harness to measure per-iteration HW
    # time as a slope, cancelling dispatch overhead. Iterations are
    # serialized (DVE waits for the previous iteration's scatters) so the
    # slope matches the one-shot latency of the full compute+scatter.
    gj = J // ng
    assert gj * ng == J
    nc = bass.Bass("TRN2")
    xs_d = nc.dram_tensor("xs", [P, J], F32, kind="ExternalInput")
    ib_d = nc.dram_tensor("ibase", [P, J], I32, kind="ExternalInput")
    out_d = nc.dram_tensor("out", [R, N], F32, kind="ExternalOutput")

    Bc = np.asarray(B_np, dtype=np.float64)  # [Q+1, Q+1], ascending powers

    with (
        nc.sbuf_tensor("xs_t", [P, J], F32) as xs_t,
        nc.sbuf_tensor("ib_t", [P, J], I32) as ib_t,
        nc.sbuf_tensor("fi_f", [P, J], F32) as fi_f,
        nc.sbuf_tensor("gt_t", [P, J], F32) as gt_t,
        nc.sbuf_tensor("frac", [P, J], F32) as frac,
        nc.sbuf_tensor("xl", [P, J], F32) as xl,
        nc.sbuf_tensor("hh", [P, J], F32) as hh,
        nc.sbuf_tensor("fi_i", [P, J], I32) as fi_i,
        nc.sbuf_tensor("idx", [P, J], I32) as idx,
        nc.sbuf_tensor("vals", [P, (Q + 1) * J], F32) as vals,
        nc.semaphore("xsem") as xsem,
        nc.semaphore("csem") as csem,
        nc.semaphore("ssem") as ssem,
        nc.semaphore("vsem") as vsem,
    ):
        with nc.Block() as block:

            @block.vector
            def _(v):
                # DVE ops are chained through vsem: deep engine pipelines
                # mean same-engine RAW hazards still need semaphore sync.
                nv = 0

                def step(inst):
                    nonlocal nv
                    inst.then_inc(vsem, 1)
                    nv += 1

                def fence():
                    v.wait_ge(vsem, nv)

                v.wait_ge(xsem, 32)
                for it in range(iters):
                    if it:
                        # scatter of iter it-1 still reads vals/idx: wait
                        # before overwriting (same values, but keep the
                        # race detector and HW ordering clean).
                        v.wait_ge(ssem, 16 * ng * it)
                    # first_i = floor(xs) for xs >= 0, robust to any
                    # f32->i32 rounding mode: convert, round-trip,
                    # subtract 1 where the round-trip exceeded xs.
                    step(v.tensor_copy(out=fi_i[:], in_=xs_t[:]))
                    fence()
                    step(v.tensor_copy(out=fi_f[:], in_=fi_i[:]))
                    fence()
                    step(v.tensor_tensor(out=gt_t[:], in0=fi_f[:],
                                         in1=xs_t[:], op=ALU.is_gt))
                    fence()
                    step(v.tensor_tensor(out=fi_f[:], in0=fi_f[:],
                                         in1=gt_t[:], op=ALU.subtract))
                    fence()
                    step(v.tensor_tensor(out=frac[:], in0=xs_t[:],
                                         in1=fi_f[:], op=ALU.subtract))
                    step(v.tensor_copy(out=fi_i[:], in_=fi_f[:]))
                    fence()
                    # idx = ibase + first_i, ibase = ((j%GJ)*P + p)*N so
                    # every value stays < 2^23 (f32-ALU exact).
                    step(v.tensor_tensor(out=idx[:], in0=ib_t[:],
                                         in1=fi_i[:], op=ALU.add))
                    # values[k] = Horner(B[k], frac + (Q-k)), interleaved
                    # so vals[p, 4j+k] = value_k(row j*128+p)
                    vv = vals[:].rearrange("p (j k) -> p j k", k=Q + 1)
                    for k in range(Q + 1):
                        b3, b2, b1, b0 = (float(Bc[k, 3]), float(Bc[k, 2]),
                                          float(Bc[k, 1]), float(Bc[k, 0]))
                        fence()
                        step(v.tensor_scalar(out=xl[:], in0=frac[:],
                                             scalar1=float(Q - k),
                                             scalar2=None, op0=ALU.add))
                        fence()
                        step(v.tensor_scalar(out=hh[:], in0=xl[:], scalar1=b3,
                                             scalar2=b2,
                                             op0=ALU.mult, op1=ALU.add))
                        fence()
                        step(v.tensor_tensor(out=hh[:], in0=hh[:], in1=xl[:],
                                             op=ALU.mult))
                        fence()
                        step(v.tensor_scalar(out=hh[:], in0=hh[:], scalar1=b1,
                                             scalar2=None, op0=ALU.add))
                        fence()
                        step(v.tensor_tensor(out=hh[:], in0=hh[:], in1=xl[:],
                                             op=ALU.mult))
                        fence()
                        step(v.tensor_scalar(out=vv[:, :, k], in0=hh[:],
                                             scalar1=b0,
                                             scalar2=None, op0=ALU.add))
                    fence()
                    v.sem_inc(csem, 1)

            @block.gpsimd
            def _(g):
                g.dma_start(out=xs_t[:], in_=xs_d[:]).then_inc(xsem, 16)
                g.dma_start(out=ib_t[:], in_=ib_d[:]).then_inc(xsem, 16)
                for it in range(iters):
                    g.wait_ge(csem, it + 1)
                    for gi in range(ng):
                        # P*GJ offsets, 16 B payload per offset: writes
                        # vals[p, 4j:4j+4] at element
                        # g*GJ*P*N + idx[p, j] = (j*P + p)*N + first_i
                        g.indirect_dma_start(
                            out=out_d[:],
                            out_offset=bass.IndirectOffsetOnAxis(
                                ap=idx[:, gi * gj:(gi + 1) * gj], axis=1),
                            in_=vals[:, (Q + 1) * gj * gi:
                                     (Q + 1) * gj * (gi + 1)],
                            in_offset=None,
                            element_offset=gi * gj * P * N,
                        ).then_inc(ssem, 16)
                g.wait_ge(ssem, 16 * ng * iters)

    return nc


_CACHE: dict[bytes, bass.Bass] = {}


def _get_program(B: np.ndarray) -> bass.Bass:
    key = np.asarray(B, dtype=np.float32).tobytes()
    if key not in _CACHE:
        _CACHE[key] = _build(B)
    return _CACHE[key]


def _in_maps(xs: np.ndarray) -> list[dict[str, np.ndarray]]:
    # j-major row layout: xs2d[p, j] = xs_shard[j*P + p]; the row base
    # offset within a scatter group is ((j%GJ)*P + p)*N (< 2^23 so DVE
    # f32-ALU int math is exact); the group base g*GJ*P*N goes in via
    # indirect-DMA element_offset.
    jj = np.arange(J, dtype=np.int32)
    pp = np.arange(P, dtype=np.int32)
    ibase = ((jj[None, :] % GJ) * P + pp[:, None]) * N
    ibase = np.ascontiguousarray(ibase.astype(np.int32))
    maps = []
    for c in range(NCORES):
        shard = np.asarray(xs[c * R:(c + 1) * R], dtype=np.float32)
        xs2d = np.ascontiguousarray(shard.reshape(J, P).T)
        maps.append({"xs": xs2d, "ibase": ibase})
    return maps


def kernel(xs, B, n, q):
    xs = np.asarray(xs, dtype=np.float32)
    B = np.asarray(B, dtype=np.float32)
    n = int(np.asarray(n)) if not isinstance(n, int) else n
    q = int(np.asarray(q)) if not isinstance(q, int) else q
    assert xs.shape == (NS,), xs.shape
    assert B.shape == (Q + 1, Q + 1), B.shape
    assert n == N and q == Q, (n, q)

    nc = _get_program(B)
    try:
        res = run_bass_kernel_spmd(nc, _in_maps(xs), core_ids=list(range(NCORES)))
    except Exception:
        # one retry for transient device-state errors (e.g. a wedged core
        # left over from a previous process)
        res = run_bass_kernel_spmd(nc, _in_maps(xs), core_ids=list(range(NCORES)))
    return np.concatenate([res.results[c]["out"] for c in range(NCORES)], axis=0)
